# revision 15
# baseline (speedup 1.0000x reference)
"""Causal self-attention (B=2, T=4096, C=768, H=12, D=64) on 8 Trainium2 cores.

Sharding: 2 batches x 4 head-groups (3 heads each). Per core:
  - qkv projection for its 3 heads, computed in transposed layout [dim, T]
  - flash-style causal attention per head (no running max: scores are O(1))
  - row-parallel output projection partial [T, C]
  - ReduceScatter(add) over the 4 cores of the same batch -> [T/4, C] slice

Matmul operands are bf16 (fp32 PSUM accumulation); host pre-casts x and the
weight slices. Host gathers the 8 [1024, 768] slices into [2, 4096, 768].
"""

import sys

sys.path.insert(0, "/opt/trn_rl_repo")

import numpy as np
import ml_dtypes

import concourse.bass as bass
import concourse.tile as tile
from concourse import bacc, mybir
from concourse.bass import ds
from concourse.bass_utils import run_bass_kernel_spmd
from concourse.masks import make_identity

T = 4096
C = 768
D = 64
NCORES = 8
G = 4  # cores per batch (head-groups)
HPC = 3  # heads per core
TSL = T // G  # output token slice per core
QC = 512  # q-chunk (free dim of S^T matmuls)
NQC = T // QC
F32 = mybir.dt.float32
BF16 = mybir.dt.bfloat16
FX = mybir.ActivationFunctionType

NEG = -1.0e9


def _body(ctx, tc, collective=True):
    nc = tc.nc
    mm = nc.tensor.matmul
    xb = nc.dram_tensor("xb", [T, C], BF16, kind="ExternalInput").ap()
    wc = nc.dram_tensor("wc", [C, 576], BF16, kind="ExternalInput").ap()
    bc = nc.dram_tensor("bc", [576], F32, kind="ExternalInput").ap()
    wp = nc.dram_tensor("wp", [193, C], BF16, kind="ExternalInput").ap()
    outp = nc.dram_tensor("outp", [TSL, C], F32, kind="ExternalOutput").ap()
    partial = nc.dram_tensor("partial", [T, C], F32).ap()
    rsout = nc.dram_tensor("rsout", [TSL, C], F32).ap()

    cp = ctx.enter_context(tc.tile_pool(name="consts", bufs=1))
    mp = ctx.enter_context(tc.tile_pool(name="main", bufs=1))

    ident = cp.tile([128, 128], BF16)
    make_identity(nc, ident[:])
    masks = cp.tile([128, 4, QC], F32)
    for r in range(4):
        nc.gpsimd.memset(masks[:, r, :], 0.0)
        # keep 0 where (j - p - 128r) >= 0 i.e. kpos <= qpos; else fill NEG
        nc.gpsimd.affine_select(
            out=masks[:, r, :],
            in_=masks[:, r, :],
            compare_op=mybir.AluOpType.is_ge,
            fill=NEG,
            base=-128 * r,
            pattern=[[1, QC]],
            channel_multiplier=-1,
        )
    onesT = cp.tile([65, 64], BF16)
    nc.gpsimd.memset(onesT[:], 1.0)
    bcol = cp.tile([128, 5], F32)
    for m in range(4):
        nc.sync.dma_start(bcol[:, m : m + 1], bc[ds(128 * m, 128)])
    nc.sync.dma_start(bcol[0:64, 4:5], bc[ds(512, 64)])
    wpa = cp.tile([64, C], BF16)
    wpb = cp.tile([64, C], BF16)
    wpc = cp.tile([65, C], BF16)
    nc.sync.dma_start(wpa[:], wp[0:64, :])
    nc.sync.dma_start(wpb[:], wp[64:128, :])
    nc.sync.dma_start(wpc[:], wp[128:193, :])

    # qkvT partition-tiles (columns of wc, order fixed host-side):
    #   m=0: [q_h0 | q_h1]   m=1: [k_h0 | k_h1]   m=2: [v_h0 | v_h1]
    #   m=3: [q_h2 | v_h2]   m=4: [k_h2 | -]
    xT = mp.tile([128, 6, T], BF16)
    qkvT = mp.tile([128, 5, T], BF16)
    vaug = mp.tile([128, T // 128, 3 * 65], BF16)
    yt0 = mp.tile([64, T], BF16)
    yt1 = mp.tile([64, T], BF16)
    yt2 = mp.tile([65, T], BF16)  # row 64 = ones (bias row for proj)
    nc.gpsimd.memset(yt2[64:65, :], 1.0)

    qT = [qkvT[0:64, 0], qkvT[64:128, 0], qkvT[0:64, 3]]
    kT = [qkvT[0:64, 1], qkvT[64:128, 1], qkvT[0:64, 4]]
    yt = [yt0[:], yt1[:], yt2[0:64]]
    msizes = [128, 128, 128, 128, 64]

    # PSUM budget (8 banks): mm(2) + ps2(2x2) + ya(2) = 8
    with (
        tc.tile_pool(name="wst", bufs=1) as wstp,
        tc.tile_pool(name="ex", bufs=6) as exp_,
        tc.tile_pool(name="rd", bufs=2) as rdp,
        tc.tile_pool(name="prt", bufs=3) as prtp,
        tc.tile_pool(name="mmp", bufs=2, space="PSUM") as mmp,
        tc.tile_pool(name="ps2", bufs=2, space="PSUM") as ps2p,
        tc.tile_pool(name="tp", bufs=2, space="PSUM") as tpp,
    ):
        wst = wstp.tile([128, 6, 576], BF16)
        nc.sync.dma_start(wst[:], wc.rearrange("(kc p) d -> p kc d", p=128))
        # x^T via hardware DMA transpose (xbar), per (token-block, C-chunk)
        for nb in range(NQC):
            for kc in range(6):
                nc.sync.dma_start(
                    xT[:, kc, ds(QC * nb, QC)],
                    xb[ds(QC * nb, QC), :][:, ds(128 * kc, 128)],
                    transpose=True,
                )

        # ---- phase 1: qkv projection + v_aug (per 512-token block) ----
        for nb in range(NQC):
            for m in range(5):
                msz = msizes[m]
                psq = mmp.tile([128, QC], F32, tag="mm")
                for kc in range(6):
                    mm(
                        psq[0:msz, :],
                        wst[:, kc, ds(128 * m, msz)],
                        xT[:, kc, ds(QC * nb, QC)],
                        start=(kc == 0),
                        stop=(kc == 5),
                    )
                nc.vector.tensor_scalar_add(
                    qkvT[0:msz, m, ds(QC * nb, QC)],
                    psq[0:msz, :],
                    bcol[0:msz, m : m + 1],
                )
            # v_aug for this block's 4 token tiles via PE transpose
            for tt in range(4 * nb, 4 * nb + 4):
                psv = tpp.tile([128, 128], BF16, tag="tp")
                nc.tensor.transpose(
                    psv[:], qkvT[:, 2, ds(128 * tt, 128)], ident[:]
                )
                nc.vector.tensor_copy(
                    vaug[:, tt, :].rearrange("p (h c) -> p h c", c=65)[:, 0:2, 0:64],
                    psv.rearrange("p (h c) -> p h c", c=64),
                )
                psv2 = tpp.tile([128, 128], BF16, tag="tp")
                nc.tensor.transpose(
                    psv2[0:128, 0:64],
                    qkvT[64:128, 3, ds(128 * tt, 128)],
                    ident[64:128, 64:128],
                )
                nc.vector.tensor_copy(vaug[:, tt, 130:194], psv2[0:128, 0:64])
                nc.vector.memset(
                    vaug[:, tt, :].rearrange("p (h c) -> p h c", c=65)[:, :, 64:65],
                    1.0,
                )

        # ---- phase 2: attention (qc-outer) + interleaved output projection ----
        def proj_tile(tt):
            prt = prtp.tile([128, C], F32)
            for nn in range(2):
                psp = mmp.tile([128, QC], F32, tag="mm")
                mm(psp[:, 0:384], yt0[:, ds(128 * tt, 128)],
                   wpa[:, ds(384 * nn, 384)], start=True, stop=False)
                mm(psp[:, 0:384], yt1[:, ds(128 * tt, 128)],
                   wpb[:, ds(384 * nn, 384)], start=False, stop=False)
                mm(psp[:, 0:384], yt2[:, ds(128 * tt, 128)],
                   wpc[:, ds(384 * nn, 384)], start=False, stop=True)
                nc.vector.tensor_copy(prt[:, ds(384 * nn, 384)], psp[:, 0:384])
            nc.sync.dma_start(partial[ds(128 * tt, 128), :], prt[:])

        for qc in range(NQC):
            for h in range(HPC):
                ngr = 2 * qc + 2  # groups of 2 k-tiles, causal
                ya = mmp.tile([128, QC], F32, tag="mm")
                for g in range(ngr):
                    ps2 = ps2p.tile([128, 2, QC], F32, tag="ps2")
                    for i in range(2):
                        kt = 2 * g + i
                        mm(
                            ps2[:, i, :],
                            kT[h][:, ds(128 * kt, 128)],
                            qT[h][:, ds(QC * qc, QC)],
                            start=True,
                            stop=True,
                        )
                    if g >= 2 * qc:  # diagonal pair: additive causal mask
                        r = 2 * (g - 2 * qc)
                        nc.vector.tensor_add(ps2[:], ps2[:], masks[:, r : r + 2, :])
                    ex = exp_.tile([128, 2, QC], BF16)
                    nc.scalar.activation(ex[:], ps2[:], FX.Exp, scale=0.125)
                    for i in range(2):
                        kt = 2 * g + i
                        mm(
                            ya[0:65, :],
                            vaug[:, kt, ds(65 * h, 65)],
                            ex[:, i, :],
                            start=(kt == 0),
                            stop=(kt == 4 * qc + 3),
                        )
                # normalize: y /= denom (denom broadcast via ones matmul)
                rd = rdp.tile([65, QC], BF16)
                with nc.allow_low_precision(reason="bf16 softmax denom recip"):
                    nc.vector.reciprocal(rd[64:65, :], ya[64:65, :])
                db = mmp.tile([128, QC], F32, tag="mm")
                mm(
                    db[0:64, :],
                    onesT[64:65, 0:64],
                    rd[64:65, :],
                    start=True,
                    stop=True,
                )
                dst = yt[h][:, ds(QC * qc, QC)]
                nc.vector.tensor_copy(dst, ya[0:64, :])
                nc.vector.tensor_mul(dst, dst, db[0:64, :])
                # interleave: project one token tile of the previous q-chunk
                if qc > 0:
                    proj_tile(4 * (qc - 1) + h)
            if qc > 0:
                proj_tile(4 * (qc - 1) + 3)
        for tt in range(4 * (NQC - 1), T // 128):
            proj_tile(tt)

    # ---- phase 5: ReduceScatter over the batch's 4 cores, emit slice ----
    if collective:
        nc.gpsimd.collective_compute(
            "ReduceScatter",
            mybir.AluOpType.add,
            replica_groups=[[0, 1, 2, 3], [4, 5, 6, 7]],
            ins=[partial.opt()],
            outs=[rsout.opt()],
        )
        nc.sync.dma_start(outp[:], rsout[:])
    else:
        nc.sync.dma_start(outp[:], partial[0:TSL, :])


_PROGRAM = None


def build_program(collective=True):
    global _PROGRAM
    if collective and _PROGRAM is not None:
        return _PROGRAM
    from contextlib import ExitStack

    nc = bacc.Bacc(
        trn_type="TRN2",
        target_bir_lowering=False,
        debug=False,
        num_devices=NCORES if collective else 1,
    )
    with tile.TileContext(nc) as tc:
        with ExitStack() as ctx:
            _body(ctx, tc, collective=collective)
    nc.compile()
    if collective:
        _PROGRAM = nc
    return nc


def make_in_maps(x, Wqkv, bqkv, Wproj, bproj):
    x = np.asarray(x, dtype=np.float32)
    Wqkv = np.asarray(Wqkv, dtype=np.float32)
    bqkv = np.asarray(bqkv, dtype=np.float32)
    Wproj = np.asarray(Wproj, dtype=np.float32)
    bproj = np.asarray(bproj, dtype=np.float32)
    bf = ml_dtypes.bfloat16

    in_maps = []
    for c in range(NCORES):
        b, g = divmod(c, G)
        h = [3 * g + j for j in range(HPC)]  # global head ids
        qs = [Wqkv[:, 64 * hh : 64 * hh + 64] for hh in h]
        ks = [Wqkv[:, C + 64 * hh : C + 64 * hh + 64] for hh in h]
        vs = [Wqkv[:, 2 * C + 64 * hh : 2 * C + 64 * hh + 64] for hh in h]
        wcc = np.concatenate(
            [qs[0], qs[1], ks[0], ks[1], vs[0], vs[1], qs[2], vs[2], ks[2]], axis=1
        )
        bq = [bqkv[64 * hh : 64 * hh + 64] for hh in h]
        bk = [bqkv[C + 64 * hh : C + 64 * hh + 64] for hh in h]
        bv = [bqkv[2 * C + 64 * hh : 2 * C + 64 * hh + 64] for hh in h]
        bcc = np.concatenate(
            [bq[0], bq[1], bk[0], bk[1], bv[0], bv[1], bq[2], bv[2], bk[2]]
        )
        wprows = np.concatenate(
            [Wproj[64 * hh : 64 * hh + 64, :] for hh in h]
            + [(bproj if g == 0 else np.zeros_like(bproj))[None, :]],
            axis=0,
        )
        in_maps.append(
            {
                "xb": np.ascontiguousarray(x[b]).astype(bf),
                "wc": np.ascontiguousarray(wcc).astype(bf),
                "bc": np.ascontiguousarray(bcc),
                "wp": np.ascontiguousarray(wprows).astype(bf),
            }
        )
    return in_maps


def kernel(x, Wqkv, bqkv, Wproj, bproj):
    nc = build_program()
    in_maps = make_in_maps(x, Wqkv, bqkv, Wproj, bproj)
    res = run_bass_kernel_spmd(nc, in_maps, list(range(NCORES)))
    out = np.empty((2, T, C), dtype=np.float32)
    for c in range(NCORES):
        b, g = divmod(c, G)
        out[b, TSL * g : TSL * (g + 1), :] = res.results[c]["outp"]
    return out


# revision 17
# speedup vs baseline: 4672.3845x; 4672.3845x over previous
"""Causal self-attention (B=2, T=4096, C=768, H=12, D=64) on 8 Trainium2 cores.

Sharding: 2 batches x 4 head-groups (3 heads each). Per core:
  - qkv projection for its 3 heads, computed in transposed layout [dim, T]
  - flash-style causal attention per head (no running max: scores are O(1))
  - row-parallel output projection partial [T, C]
  - ReduceScatter(add) over the 4 cores of the same batch -> [T/4, C] slice

Matmul operands are bf16 (fp32 PSUM accumulation); host pre-casts x and the
weight slices. Host gathers the 8 [1024, 768] slices into [2, 4096, 768].
"""

import sys

sys.path.insert(0, "/opt/trn_rl_repo")

import numpy as np
import ml_dtypes

import concourse.bass as bass
import concourse.tile as tile
from concourse import bacc, mybir
from concourse.bass import ds
from concourse.bass_utils import run_bass_kernel_spmd
from concourse.masks import make_identity

T = 4096
C = 768
D = 64
NCORES = 8
G = 4  # cores per batch (head-groups)
HPC = 3  # heads per core
TSL = T // G  # output token slice per core
QC = 512  # q-chunk (free dim of S^T matmuls)
NQC = T // QC
F32 = mybir.dt.float32
BF16 = mybir.dt.bfloat16
FX = mybir.ActivationFunctionType

NEG = -1.0e9


def _body(ctx, tc, collective=True):
    nc = tc.nc
    mm = nc.tensor.matmul
    xb = nc.dram_tensor("xb", [T, C], BF16, kind="ExternalInput").ap()
    wc = nc.dram_tensor("wc", [C, 576], BF16, kind="ExternalInput").ap()
    bc = nc.dram_tensor("bc", [576], F32, kind="ExternalInput").ap()
    wp = nc.dram_tensor("wp", [193, C], BF16, kind="ExternalInput").ap()
    outp = nc.dram_tensor("outp", [TSL, C], F32, kind="ExternalOutput").ap()
    partial = nc.dram_tensor("partial", [T, C], F32).ap()
    rsout = nc.dram_tensor("rsout", [TSL, C], F32).ap()

    cp = ctx.enter_context(tc.tile_pool(name="consts", bufs=1))
    mp = ctx.enter_context(tc.tile_pool(name="main", bufs=1))

    ident = cp.tile([128, 128], BF16)
    make_identity(nc, ident[:])
    masks = cp.tile([128, 4, QC], F32)
    for r in range(4):
        nc.gpsimd.memset(masks[:, r, :], 0.0)
        # keep 0 where (j - p - 128r) >= 0 i.e. kpos <= qpos; else fill NEG
        nc.gpsimd.affine_select(
            out=masks[:, r, :],
            in_=masks[:, r, :],
            compare_op=mybir.AluOpType.is_ge,
            fill=NEG,
            base=-128 * r,
            pattern=[[1, QC]],
            channel_multiplier=-1,
        )
    onesT = cp.tile([65, 64], BF16)
    nc.gpsimd.memset(onesT[:], 1.0)
    bcol = cp.tile([128, 5], F32)
    for m in range(4):
        nc.sync.dma_start(bcol[:, m : m + 1], bc[ds(128 * m, 128)])
    nc.sync.dma_start(bcol[0:64, 4:5], bc[ds(512, 64)])
    wpa = cp.tile([64, C], BF16)
    wpb = cp.tile([64, C], BF16)
    wpc = cp.tile([65, C], BF16)
    nc.sync.dma_start(wpa[:], wp[0:64, :])
    nc.sync.dma_start(wpb[:], wp[64:128, :])
    nc.sync.dma_start(wpc[:], wp[128:193, :])

    # qkvT partition-tiles (columns of wc, order fixed host-side):
    #   m=0: [q_h0 | q_h1]   m=1: [k_h0 | k_h1]   m=2: [v_h0 | v_h1]
    #   m=3: [q_h2 | v_h2]   m=4: [k_h2 | -]
    xT = mp.tile([128, 6, T], BF16)
    qkvT = mp.tile([128, 5, T], BF16)
    vaug = mp.tile([128, T // 128, 3 * 65], BF16)
    yt0 = mp.tile([64, T], BF16)
    yt1 = mp.tile([64, T], BF16)
    yt2 = mp.tile([65, T], BF16)  # row 64 = ones (bias row for proj)
    nc.gpsimd.memset(yt2[64:65, :], 1.0)

    qT = [qkvT[0:64, 0], qkvT[64:128, 0], qkvT[0:64, 3]]
    kT = [qkvT[0:64, 1], qkvT[64:128, 1], qkvT[0:64, 4]]
    yt = [yt0[:], yt1[:], yt2[0:64]]
    msizes = [128, 128, 128, 128, 64]

    # PSUM budget (8 banks): mm(2) + ps2(2x2) + ya(2) = 8
    with (
        tc.tile_pool(name="wst", bufs=1) as wstp,
        tc.tile_pool(name="ex", bufs=6) as exp_,
        tc.tile_pool(name="rd", bufs=2) as rdp,
        tc.tile_pool(name="prt", bufs=3) as prtp,
        tc.tile_pool(name="mmp", bufs=2, space="PSUM") as mmp,
        tc.tile_pool(name="ps2", bufs=2, space="PSUM") as ps2p,
        tc.tile_pool(name="tp", bufs=2, space="PSUM") as tpp,
    ):
        wst = wstp.tile([128, 6, 576], BF16)
        nc.sync.dma_start(wst[:], wc.rearrange("(kc p) d -> p kc d", p=128))
        # x^T via hardware DMA transpose (xbar), per (token-block, C-chunk)
        for nb in range(NQC):
            for kc in range(6):
                nc.sync.dma_start(
                    xT[:, kc, ds(QC * nb, QC)],
                    xb[ds(QC * nb, QC), :][:, ds(128 * kc, 128)],
                    transpose=True,
                )

        # ---- phase 1: qkv projection + v_aug (per 512-token block) ----
        for nb in range(NQC):
            for m in range(5):
                msz = msizes[m]
                psq = mmp.tile([128, QC], F32, tag="mm")
                for kc in range(6):
                    mm(
                        psq[0:msz, :],
                        wst[:, kc, ds(128 * m, msz)],
                        xT[:, kc, ds(QC * nb, QC)],
                        start=(kc == 0),
                        stop=(kc == 5),
                    )
                nc.vector.tensor_scalar_add(
                    qkvT[0:msz, m, ds(QC * nb, QC)],
                    psq[0:msz, :],
                    bcol[0:msz, m : m + 1],
                )
            # v_aug for this block's 4 token tiles via PE transpose
            for tt in range(4 * nb, 4 * nb + 4):
                psv = tpp.tile([128, 128], BF16, tag="tp")
                nc.tensor.transpose(
                    psv[:], qkvT[:, 2, ds(128 * tt, 128)], ident[:]
                )
                nc.vector.tensor_copy(
                    vaug[:, tt, :].rearrange("p (h c) -> p h c", c=65)[:, 0:2, 0:64],
                    psv.rearrange("p (h c) -> p h c", c=64),
                )
                psv2 = tpp.tile([128, 128], BF16, tag="tp")
                nc.tensor.transpose(
                    psv2[0:128, 0:64],
                    qkvT[64:128, 3, ds(128 * tt, 128)],
                    ident[64:128, 64:128],
                )
                nc.vector.tensor_copy(vaug[:, tt, 130:194], psv2[0:128, 0:64])
                nc.vector.memset(
                    vaug[:, tt, :].rearrange("p (h c) -> p h c", c=65)[:, :, 64:65],
                    1.0,
                )

        # ---- phase 2: attention (qc-outer) + interleaved output projection ----
        def proj_tile(tt):
            prt = prtp.tile([128, C], F32)
            for nn in range(2):
                psp = mmp.tile([128, QC], F32, tag="mm")
                mm(psp[:, 0:384], yt0[:, ds(128 * tt, 128)],
                   wpa[:, ds(384 * nn, 384)], start=True, stop=False)
                mm(psp[:, 0:384], yt1[:, ds(128 * tt, 128)],
                   wpb[:, ds(384 * nn, 384)], start=False, stop=False)
                mm(psp[:, 0:384], yt2[:, ds(128 * tt, 128)],
                   wpc[:, ds(384 * nn, 384)], start=False, stop=True)
                nc.vector.tensor_copy(prt[:, ds(384 * nn, 384)], psp[:, 0:384])
            nc.sync.dma_start(partial[ds(128 * tt, 128), :], prt[:])

        for qc in range(NQC):
            for h in range(HPC):
                ngr = 2 * qc + 2  # groups of 2 k-tiles, causal
                ya = mmp.tile([128, QC], F32, tag="mm")
                for g in range(ngr):
                    ps2 = ps2p.tile([128, 2, QC], F32, tag="ps2")
                    for i in range(2):
                        kt = 2 * g + i
                        mm(
                            ps2[:, i, :],
                            kT[h][:, ds(128 * kt, 128)],
                            qT[h][:, ds(QC * qc, QC)],
                            start=True,
                            stop=True,
                        )
                    if g >= 2 * qc:  # diagonal pair: additive causal mask
                        r = 2 * (g - 2 * qc)
                        nc.vector.tensor_add(ps2[:], ps2[:], masks[:, r : r + 2, :])
                    ex = exp_.tile([128, 2, QC], BF16)
                    nc.scalar.activation(ex[:], ps2[:], FX.Exp, scale=0.125)
                    for i in range(2):
                        kt = 2 * g + i
                        mm(
                            ya[0:65, :],
                            vaug[:, kt, ds(65 * h, 65)],
                            ex[:, i, :],
                            start=(kt == 0),
                            stop=(kt == 4 * qc + 3),
                        )
                # normalize: y /= denom (denom broadcast via ones matmul)
                rd = rdp.tile([65, QC], BF16)
                with nc.allow_low_precision(reason="bf16 softmax denom recip"):
                    nc.vector.reciprocal(rd[64:65, :], ya[64:65, :])
                db = mmp.tile([128, QC], F32, tag="mm")
                mm(
                    db[0:64, :],
                    onesT[64:65, 0:64],
                    rd[64:65, :],
                    start=True,
                    stop=True,
                )
                dst = yt[h][:, ds(QC * qc, QC)]
                nc.vector.tensor_copy(dst, ya[0:64, :])
                nc.vector.tensor_mul(dst, dst, db[0:64, :])
                # interleave: project one token tile of the previous q-chunk
                if qc > 0:
                    proj_tile(4 * (qc - 1) + h)
            if qc > 0:
                proj_tile(4 * (qc - 1) + 3)
        for tt in range(4 * (NQC - 1), T // 128):
            proj_tile(tt)

    # ---- phase 5: ReduceScatter over the batch's 4 cores, emit slice ----
    if collective:
        nc.gpsimd.collective_compute(
            "ReduceScatter",
            mybir.AluOpType.add,
            replica_groups=[[0, 1, 2, 3], [4, 5, 6, 7]],
            ins=[partial.opt()],
            outs=[rsout.opt()],
        )
        nc.sync.dma_start(outp[:], rsout[:])
    else:
        nc.sync.dma_start(outp[:], partial[0:TSL, :])


_PROGRAM = None


def build_program(collective=True):
    global _PROGRAM
    if collective and _PROGRAM is not None:
        return _PROGRAM
    from contextlib import ExitStack

    nc = bacc.Bacc(
        trn_type="TRN2",
        target_bir_lowering=False,
        debug=False,
        num_devices=NCORES if collective else 1,
    )
    with tile.TileContext(nc) as tc:
        with ExitStack() as ctx:
            _body(ctx, tc, collective=collective)
    nc.compile()
    if collective:
        _PROGRAM = nc
    return nc


def make_in_maps(x, Wqkv, bqkv, Wproj, bproj):
    x = np.asarray(x, dtype=np.float32)
    Wqkv = np.asarray(Wqkv, dtype=np.float32)
    bqkv = np.asarray(bqkv, dtype=np.float32)
    Wproj = np.asarray(Wproj, dtype=np.float32)
    bproj = np.asarray(bproj, dtype=np.float32)
    bf = ml_dtypes.bfloat16

    in_maps = []
    for c in range(NCORES):
        b, g = divmod(c, G)
        h = [3 * g + j for j in range(HPC)]  # global head ids
        qs = [Wqkv[:, 64 * hh : 64 * hh + 64] for hh in h]
        ks = [Wqkv[:, C + 64 * hh : C + 64 * hh + 64] for hh in h]
        vs = [Wqkv[:, 2 * C + 64 * hh : 2 * C + 64 * hh + 64] for hh in h]
        wcc = np.concatenate(
            [qs[0], qs[1], ks[0], ks[1], vs[0], vs[1], qs[2], vs[2], ks[2]], axis=1
        )
        bq = [bqkv[64 * hh : 64 * hh + 64] for hh in h]
        bk = [bqkv[C + 64 * hh : C + 64 * hh + 64] for hh in h]
        bv = [bqkv[2 * C + 64 * hh : 2 * C + 64 * hh + 64] for hh in h]
        bcc = np.concatenate(
            [bq[0], bq[1], bk[0], bk[1], bv[0], bv[1], bq[2], bv[2], bk[2]]
        )
        wprows = np.concatenate(
            [Wproj[64 * hh : 64 * hh + 64, :] for hh in h]
            + [(bproj if g == 0 else np.zeros_like(bproj))[None, :]],
            axis=0,
        )
        in_maps.append(
            {
                "xb": np.ascontiguousarray(x[b]).astype(bf),
                "wc": np.ascontiguousarray(wcc).astype(bf),
                "bc": np.ascontiguousarray(bcc),
                "wp": np.ascontiguousarray(wprows).astype(bf),
            }
        )
    return in_maps


def kernel(x, Wqkv, bqkv, Wproj, bproj):
    nc = build_program()
    in_maps = make_in_maps(x, Wqkv, bqkv, Wproj, bproj)
    res = run_bass_kernel_spmd(nc, in_maps, list(range(NCORES)))
    out = np.empty((2, T, C), dtype=np.float32)
    for c in range(NCORES):
        b, g = divmod(c, G)
        out[b, TSL * g : TSL * (g + 1), :] = res.results[c]["outp"]
    return out
